# revision 32
# baseline (speedup 1.0000x reference)
"""GATv2 message-passing kernel for 8 Trainium2 NeuronCores (Bass/Tile).

Strategy (edge parallelism over receiver-sorted edges):
  * Sort edges by receiver on the host; receivers fall into 128-node blocks.
  * Deal the blocks to the 8 cores balanced by edge count, so every core owns
    complete receiver segments -> the segment softmax/sums are core-local and
    no collective is needed.  Each core returns its blocks' output rows and
    the host reassembles them.
  * On device, per core: phase A computes h = node_features @ W (bias folded
    out algebraically) into an HBM scratch; phase B streams 512-edge chunks:
    indirect-DMA gather of h[senders], one-hot matrices built with is_equal
    compares, and all adds/transposes/segment-sums done as PE matmuls
    accumulating in PSUM.  Mish/Exp run on the scalar (ACT) engine.
  * Softmax is computed without the segment-max shift: logits here are
    bounded (|logit| < ~15), so exp() cannot overflow fp32 and the result is
    mathematically identical.
"""

import numpy as np

import concourse.bass as bass
import concourse.bacc as bacc
import concourse.tile as tile
from concourse import mybir
from concourse.bass_utils import run_bass_kernel_spmd

F32 = mybir.dt.float32
I32 = mybir.dt.int32
AF = mybir.ActivationFunctionType
OP = mybir.AluOpType

N_NODES = 50000
N_EDGES = 800000
IN_DIM = 256
EDGE_DIM = 64
EMBED = 128
HEADS = 8
HEAD_DIM = EMBED // HEADS
P = 128
NCORES = 8
CHUNK_G = 4  # groups (of 128 edges) per processing chunk
PAD_RLOC = 200.0  # sentinel receiver-local id for padding edges (> 127)


# ---------------------------------------------------------------- host plan

def _plan(receivers, senders, n_nodes, ncores):
    """Sort edges by receiver, then by sender within each 128-node receiver
    block (so gathers use monotone addresses and fit int16 index windows);
    deal blocks to cores balanced by edge count; pad every (core, position)
    to a common group count; pick per-(position, chunk) gather base offsets
    shared by all cores."""
    order = np.argsort(receivers, kind="stable").astype(np.int64)
    r_s = receivers[order].astype(np.int64)
    nb = -(-n_nodes // P)
    npos = -(-nb // ncores)
    nb_pad = npos * ncores
    n_pad = nb_pad * P
    cnt = np.bincount(r_s // P, minlength=nb_pad).astype(np.int64)
    estart = np.zeros(nb_pad, np.int64)
    estart[1:] = np.cumsum(cnt)[:-1]
    # sender-sort within each receiver block
    for b in range(nb_pad):
        e0, c = int(estart[b]), int(cnt[b])
        if c > 1:
            seg = order[e0:e0 + c]
            order[e0:e0 + c] = seg[np.argsort(senders[seg], kind="stable")]
    r_s = receivers[order].astype(np.int64)
    gcnt = np.maximum(-(-cnt // P), 1)
    deal = np.argsort(-gcnt, kind="stable")
    blocks = deal.reshape(npos, ncores)  # blocks[pos, core] -> block id
    gpos = gcnt[blocks].max(axis=1)      # groups per position (same all cores)
    goff = np.zeros(npos, np.int64)
    goff[1:] = np.cumsum(gpos)[:-1]
    # per-(position, chunk) gather windows, uniform across cores
    bases = []
    for pos in range(npos):
        gp = int(gpos[pos])
        nch = -(-gp // CHUNK_G)
        lo = np.full(nch, np.iinfo(np.int64).max)
        hi = np.zeros(nch, np.int64)
        for core in range(ncores):
            b = int(blocks[pos, core])
            e0, c = int(estart[b]), int(cnt[b])
            sc = senders[order[e0:e0 + c]].astype(np.int64)
            for ch in range(nch):  # real edges only; pads gather row `base`
                part = sc[ch * CHUNK_G * P:(ch + 1) * CHUNK_G * P]
                if part.size:
                    lo[ch] = min(lo[ch], int(part.min()))
                    hi[ch] = max(hi[ch], int(part.max()))
        lo[lo > hi] = 0  # all-padding chunks
        assert (hi - lo).max() < 32768, \
            f"gather window overflow at pos {pos}: {(hi - lo).max()}"
        bases.append([int(x) for x in lo])
    return dict(order=order, r_s=r_s, cnt=cnt, estart=estart, blocks=blocks,
                gpos=gpos, goff=goff, gtot=int(gpos.sum()),
                ecap=int(gpos.sum()) * P, npos=npos, nb_pad=nb_pad,
                bases=bases, n_pad=n_pad)


def _host_inputs(plan, node_features, edge_features, W_kernel, W_bias,
                 We_kernel, We_bias, a, senders):
    """Build the per-core input maps (all numpy, no math beyond transposes)."""
    npos, gtot, ecap = plan["npos"], plan["gtot"], plan["ecap"]
    n_pad = plan["nb_pad"] * P
    n_nodes, in_dim = node_features.shape
    heads, head_dim = a.shape
    embed = heads * head_dim
    edge_dim = edge_features.shape[1]

    nfT = np.zeros((in_dim, n_pad), np.float32)
    nfT[:, :n_nodes] = node_features.T
    We_aug = np.concatenate(
        [We_kernel, (We_bias + 2.0 * W_bias)[None, :]], axis=0
    ).astype(np.float32)
    A_blk = np.zeros((embed, heads), np.float32)
    for h in range(heads):
        A_blk[h * head_dim:(h + 1) * head_dim, h] = a[h]
    Wb_rep = np.tile(W_bias[None, :], (P, 1)).astype(np.float32)
    identity = np.eye(P, dtype=np.float32)
    iota_row = np.tile(np.arange(P, dtype=np.float32)[None, :], (P, 1))
    iota_col = np.arange(P, dtype=np.float32)[:, None].copy()
    ones_row = np.ones((1, P), np.float32)

    efT_all = np.ascontiguousarray(edge_features[plan["order"]].T)  # [64, E]
    s_sorted = senders[plan["order"]].astype(np.int32)
    rloc_all = (plan["r_s"] - (plan["r_s"] // P) * P).astype(np.float32)

    shared = {
        "nfT": nfT, "W": W_kernel.astype(np.float32), "We_aug": We_aug,
        "A_blk": A_blk, "Wb_rep": Wb_rep, "identity": identity,
        "iota_row": iota_row, "iota_col": iota_col, "ones_row": ones_row,
    }
    in_maps = []
    for core in range(NCORES):
        senders16 = np.zeros((P, gtot * 8), np.int16)
        rloc_col = np.full((P, gtot), PAD_RLOC, np.float32)
        rloc_flat = np.full((1, ecap), PAD_RLOC, np.float32)
        efTa = np.zeros((edge_dim + 1, ecap), np.float32)
        efTa[edge_dim, :] = 1.0
        blocknodes = np.zeros((P, npos), np.int32)
        for pos in range(npos):
            b = int(plan["blocks"][pos, core])
            g0 = int(plan["goff"][pos])
            gp = int(plan["gpos"][pos])
            c = int(plan["cnt"][b])
            e0 = int(plan["estart"][b])
            blocknodes[:, pos] = b * P + np.arange(P)
            col0 = g0 * P
            efTa[:edge_dim, col0:col0 + c] = efT_all[:, e0:e0 + c]
            rloc_flat[0, col0:col0 + c] = rloc_all[e0:e0 + c]
            nch = -(-gp // CHUNK_G)
            for ch in range(nch):
                s_ch = min(CHUNK_G * P, gp * P - ch * CHUNK_G * P)
                base = plan["bases"][pos][ch]
                tmp_s = np.full(s_ch, base, np.int64)  # pads -> row `base`
                r0 = ch * CHUNK_G * P
                nreal = min(max(c - r0, 0), s_ch)
                tmp_s[:nreal] = s_sorted[e0 + r0:e0 + r0 + nreal]
                rel = (tmp_s - base).astype(np.int16)
                blk16 = np.tile(rel.reshape(s_ch // 16, 16).T, (8, 1))
                cb = (g0 * P + ch * CHUNK_G * P) // 16
                senders16[:, cb:cb + s_ch // 16] = blk16
            tmp_r = np.full(gp * P, PAD_RLOC, np.float32)
            tmp_r[:c] = rloc_all[e0:e0 + c]
            rloc_col[:, g0:g0 + gp] = tmp_r.reshape(gp, P).T
        m = dict(shared)
        m.update({"senders16": senders16, "rloc_col": rloc_col,
                  "rloc_flat": rloc_flat, "efTa": efTa,
                  "blocknodes": blocknodes})
        in_maps.append(m)
    return in_maps


# ---------------------------------------------------------------- bass build

def _build(plan, n_pad, in_dim, edge_dim, embed, heads, debug=False):
    head_dim = embed // heads
    npos, gtot, ecap = plan["npos"], plan["gtot"], plan["ecap"]
    gpos, goff = plan["gpos"], plan["goff"]
    smax = int(gpos.max()) * P
    UW = embed + heads  # U columns: [weighted sum | denom]

    nc = bacc.Bacc("TRN2")
    t_nfT = nc.dram_tensor("nfT", [in_dim, n_pad], F32, kind="ExternalInput")
    t_W = nc.dram_tensor("W", [in_dim, embed], F32, kind="ExternalInput")
    t_We = nc.dram_tensor("We_aug", [edge_dim + 1, embed], F32,
                          kind="ExternalInput")
    t_A = nc.dram_tensor("A_blk", [embed, heads], F32, kind="ExternalInput")
    t_Wb = nc.dram_tensor("Wb_rep", [P, embed], F32, kind="ExternalInput")
    t_id = nc.dram_tensor("identity", [P, P], F32, kind="ExternalInput")
    t_ior = nc.dram_tensor("iota_row", [P, P], F32, kind="ExternalInput")
    t_ioc = nc.dram_tensor("iota_col", [P, 1], F32, kind="ExternalInput")
    t_ones = nc.dram_tensor("ones_row", [1, P], F32, kind="ExternalInput")
    t_s16 = nc.dram_tensor("senders16", [P, gtot * 8], mybir.dt.int16,
                           kind="ExternalInput")
    t_rlc = nc.dram_tensor("rloc_col", [P, gtot], F32, kind="ExternalInput")
    t_rlf = nc.dram_tensor("rloc_flat", [1, ecap], F32, kind="ExternalInput")
    t_efT = nc.dram_tensor("efTa", [edge_dim + 1, ecap], F32,
                           kind="ExternalInput")
    t_bn = nc.dram_tensor("blocknodes", [P, npos], I32, kind="ExternalInput")
    t_out = nc.dram_tensor("out", [npos * P, embed], F32,
                           kind="ExternalOutput")
    t_h = nc.dram_tensor("h_scratch", [n_pad, embed], F32, kind="Internal")
    t_dbg = None
    if debug:
        t_dbg = nc.dram_tensor("dbg", [6, P, CHUNK_G * P], F32,
                               kind="ExternalOutput")

    with tile.TileContext(nc) as tc:
        with tc.tile_pool(name="const", bufs=1) as cp:
            def cload(t, shape):
                s = cp.tile(shape, t.dtype, tag=f"c_{t.name}")
                nc.sync.dma_start(out=s[:], in_=t[:])
                return s

            W0 = cp.tile([P, embed], F32)
            nc.sync.dma_start(out=W0[:], in_=t_W[0:P, :])
            W1 = cp.tile([P, embed], F32)
            nc.sync.dma_start(out=W1[:], in_=t_W[P:2 * P, :])
            We = cload(t_We, [edge_dim + 1, embed])
            Ab = cload(t_A, [embed, heads])
            Wb = cload(t_Wb, [P, embed])
            idn = cload(t_id, [P, P])
            ior = cload(t_ior, [P, P])
            ioc = cload(t_ioc, [P, 1])
            ones = cload(t_ones, [1, P])
            s16 = cload(t_s16, [P, gtot * 8])
            rlocc = cload(t_rlc, [P, gtot])
            bn = cload(t_bn, [P, npos])

            # ---------------- phase A: h = nf @ W (no bias) ----------------
            with tc.tile_pool(name="ha", bufs=4) as hap, \
                    tc.tile_pool(name="haps", bufs=4, space="PSUM") as hpp:
                for nt in range(n_pad // P):
                    na = hap.tile([P, P], F32, tag="nfT0")
                    nc.sync.dma_start(out=na[:],
                                      in_=t_nfT[0:P, nt * P:(nt + 1) * P])
                    nb_t = hap.tile([P, P], F32, tag="nfT1")
                    nc.sync.dma_start(out=nb_t[:],
                                      in_=t_nfT[P:2 * P, nt * P:(nt + 1) * P])
                    hp = hpp.tile([P, embed], F32, tag="hps")
                    nc.tensor.matmul(hp[:], lhsT=na[:], rhs=W0[:],
                                     start=True, stop=False)
                    nc.tensor.matmul(hp[:], lhsT=nb_t[:], rhs=W1[:],
                                     start=False, stop=True)
                    hs = hap.tile([P, embed], F32, tag="hsb")
                    nc.scalar.activation(out=hs[:], in_=hp[:], func=AF.Copy)
                    nc.sync.dma_start(out=t_h[nt * P:(nt + 1) * P, :],
                                      in_=hs[:])

            tc.strict_bb_all_engine_barrier()
            if debug:
                nc.sync.dma_start(out=t_dbg[0, :, 0:embed],
                                  in_=t_h[0:P, :])

            # ---------------- phase B: edge processing ---------------------
            with tc.tile_pool(name="eb", bufs=4) as ep, \
                    tc.tile_pool(name="ebsm", bufs=4) as esm, \
                    tc.tile_pool(name="ebp", bufs=2, space="PSUM") as pp, \
                    tc.tile_pool(name="ups", bufs=2, space="PSUM") as up:
                for pos in range(npos):
                    g_here = int(gpos[pos])
                    g0 = int(goff[pos])
                    Hb = ep.tile([P, embed], F32, tag="Hb")
                    nc.gpsimd.indirect_dma_start(
                        out=Hb[:], out_offset=None, in_=t_h[:],
                        in_offset=bass.IndirectOffsetOnAxis(
                            ap=bn[:, pos:pos + 1], axis=0))
                    rrow = esm.tile([1, smax], F32, tag="rrow")
                    nc.sync.dma_start(
                        out=rrow[0:1, :g_here * P],
                        in_=t_rlf[0:1, g0 * P:(g0 + g_here) * P])
                    Ups = up.tile([P, UW], F32, tag="U")
                    lgb = up.tile([P, ((int(gpos.max()) * heads + 127) // 128)
                                   * 128], F32, tag="lgb")
                    nchunks = -(-g_here // CHUNK_G)
                    es_tiles = []
                    sp_tiles = []
                    xc_tiles = []
                    # --- stage 1: pre-activation x and softplus(x) ---------
                    # (ACT stays on the {exp, ln} table set here)
                    for c in range(nchunks):
                        gc = min(CHUNK_G, g_here - c * CHUNK_G)
                        s = gc * P
                        co = c * CHUNK_G * P       # column offset in block
                        ggl = g0 + c * CHUNK_G     # global group index
                        es = ep.tile([P, CHUNK_G * P], F32, tag="es",
                                     bufs=8)
                        es_tiles.append(es)
                        base = plan["bases"][pos][c]
                        rows = min(n_pad - base, 32768)
                        cb = g0 * 8 + c * CHUNK_G * 8
                        nc.gpsimd.dma_gather(
                            out_ap=es[:, :s].rearrange("p (j e) -> p j e",
                                                       e=embed),
                            in_ap=t_h[base:base + rows, :],
                            idxs_ap=s16[:, cb:cb + s // 16],
                            num_idxs=s, num_idxs_reg=s, elem_size=embed)
                        ef = ep.tile([edge_dim + 1, CHUNK_G * P], F32,
                                     tag="ef")
                        nc.sync.dma_start(
                            out=ef[:, :s],
                            in_=t_efT[:, g0 * P + co:g0 * P + co + s])
                        rep = pp.tile([P, CHUNK_G * P], F32, tag="rep")
                        nc.tensor.matmul(rep[:, :s], lhsT=ones[:],
                                         rhs=rrow[0:1, co:co + s],
                                         start=True, stop=True)
                        GT = ep.tile([P, CHUNK_G * P], F32, tag="GT")
                        nc.vector.tensor_scalar(
                            out=GT[:, :s], in0=rep[:, :s], scalar1=ioc[:],
                            scalar2=None, op0=OP.is_equal)
                        at = pp.tile([P, CHUNK_G * P], F32, tag="attnT")
                        nc.tensor.matmul(at[:, :s], lhsT=We[:], rhs=ef[:, :s],
                                         start=True, stop=False)
                        nc.tensor.matmul(at[:, :s], lhsT=Hb[:],
                                         rhs=GT[:, :s], start=False,
                                         stop=False)
                        for j in range(gc):
                            nc.tensor.matmul(
                                at[:, j * P:(j + 1) * P],
                                lhsT=es[:, j * P:(j + 1) * P], rhs=idn[:],
                                is_transpose=True, start=False,
                                stop=(j == gc - 1))
                        # mish(x) = x * tanh(ln(1 + exp(x))) — composed from
                        # table-mapped functions (Mish/Softplus have no
                        # compiler mapping).  xc copies x out of PSUM.
                        xc = ep.tile([P, CHUNK_G * P], F32, tag="xc", bufs=8)
                        xc_tiles.append(xc)
                        nc.vector.tensor_copy(out=xc[:, :s], in_=at[:, :s])
                        vv = ep.tile([P, CHUNK_G * P], F32, tag="vv")
                        nc.scalar.activation(out=vv[:, :s], in_=at[:, :s],
                                             func=AF.Exp)
                        sp = ep.tile([P, CHUNK_G * P], F32, tag="sp", bufs=8)
                        sp_tiles.append(sp)
                        nc.scalar.activation(out=sp[:, :s], in_=vv[:, :s],
                                             func=AF.Ln, bias=1.0)
                        if debug and pos == 0 and c == 0:
                            nc.sync.dma_start(out=t_dbg[1, :, :s],
                                              in_=es[:, :s])
                            nc.sync.dma_start(out=t_dbg[2, :, :s],
                                              in_=GT[:, :s])
                            nc.sync.dma_start(out=t_dbg[5, :, :s],
                                              in_=xc[:, :s])
                    # --- stage 2: tanh, mish, logits, block exp ------------
                    # (ACT switches to the {tanh, exp} table set)
                    for c in range(nchunks):
                        gc = min(CHUNK_G, g_here - c * CHUNK_G)
                        s = gc * P
                        mi = ep.tile([P, CHUNK_G * P], F32, tag="mish")
                        nc.scalar.activation(out=mi[:, :s],
                                             in_=sp_tiles[c][:, :s],
                                             func=AF.Tanh)
                        nc.vector.tensor_tensor(out=mi[:, :s],
                                                in0=xc_tiles[c][:, :s],
                                                in1=mi[:, :s], op=OP.mult)
                        if debug and pos == 0 and c == 0:
                            nc.sync.dma_start(out=t_dbg[3, :, :s],
                                              in_=mi[:, :s])
                        for j in range(gc):
                            nc.tensor.matmul(
                                lgb[:, (c * CHUNK_G + j) * heads:
                                    (c * CHUNK_G + j + 1) * heads],
                                lhsT=mi[:, j * P:(j + 1) * P], rhs=Ab[:],
                                start=True, stop=True)
                    exb = esm.tile([P, int(gpos.max()) * heads], F32,
                                   tag="exb")
                    nc.scalar.activation(out=exb[:, :g_here * heads],
                                         in_=lgb[:, :g_here * heads],
                                         func=AF.Exp)
                    if debug and pos == 0:
                        nc.sync.dma_start(out=t_dbg[4, :, :g_here * heads],
                                          in_=exb[:, :g_here * heads])
                    # --- stage 3: weighted scatter-accumulate ---
                    for c in range(nchunks):
                        gc = min(CHUNK_G, g_here - c * CHUNK_G)
                        s = gc * P
                        ggl = g0 + c * CHUNK_G
                        es = es_tiles[c]
                        rb = ep.tile([P, CHUNK_G * UW], F32, tag="rhsb")
                        rb3 = rb[:].rearrange("p (j c) -> p j c", j=CHUNK_G)
                        ex_view = rb3[:, :gc, embed:UW]
                        exb_view = exb[:, c * CHUNK_G * heads:
                                       (c * CHUNK_G + gc) * heads].rearrange(
                            "p (j h) -> p j h", j=gc)
                        nc.vector.tensor_copy(out=ex_view, in_=exb_view)
                        m_view = rb3[:, :gc, 0:embed].rearrange(
                            "p j (h w) -> p j h w", w=head_dim)
                        es_view = es[:, :s].rearrange(
                            "p (j h w) -> p j h w", j=gc, w=head_dim)
                        ex_b = exb_view.to_broadcast([P, gc, heads, head_dim])
                        nc.vector.tensor_tensor(out=m_view, in0=es_view,
                                                in1=ex_b, op=OP.mult)
                        for j in range(gc):
                            Gt = ep.tile([P, P], F32, tag="G")
                            nc.vector.tensor_scalar(
                                out=Gt[:], in0=ior[:],
                                scalar1=rlocc[:, ggl + j:ggl + j + 1],
                                scalar2=None, op0=OP.is_equal)
                            nc.tensor.matmul(
                                Ups[:], lhsT=Gt[:],
                                rhs=rb[:, j * UW:(j + 1) * UW],
                                start=(c == 0 and j == 0),
                                stop=(c == nchunks - 1 and j == gc - 1))
                    # ---- block epilogue: out = U / max(denom, eps) + Wb ----
                    dn = ep.tile([P, heads], F32, tag="dn")
                    nc.vector.tensor_scalar(out=dn[:],
                                            in0=Ups[:, embed:UW],
                                            scalar1=1e-30, scalar2=None,
                                            op0=OP.max)
                    rc = ep.tile([P, heads], F32, tag="rc")
                    nc.vector.reciprocal(rc[:], dn[:])
                    nd = ep.tile([P, embed], F32, tag="nodes")
                    ndv = nd[:].rearrange("p (h w) -> p h w", w=head_dim)
                    uv = Ups[:, 0:embed].rearrange("p (h w) -> p h w",
                                                   w=head_dim)
                    rcb = rc[:].to_broadcast([P, heads, head_dim])
                    nc.vector.tensor_tensor(out=ndv, in0=uv, in1=rcb,
                                            op=OP.mult)
                    nd2 = ep.tile([P, embed], F32, tag="nodes2")
                    nc.vector.tensor_tensor(out=nd2[:], in0=nd[:], in1=Wb[:],
                                            op=OP.add)
                    nc.sync.dma_start(out=t_out[pos * P:(pos + 1) * P, :],
                                      in_=nd2[:])
    nc.finalize()
    return nc


# ---------------------------------------------------------------- entry

def _run(node_features, edge_features, W_kernel, W_bias, We_kernel, We_bias,
         a, senders, receivers, trace=False):
    n_nodes, in_dim = node_features.shape
    heads, head_dim = a.shape
    embed = heads * head_dim
    edge_dim = edge_features.shape[1]
    plan = _plan(receivers, senders, n_nodes, NCORES)
    n_pad = plan["nb_pad"] * P
    in_maps = _host_inputs(plan, node_features, edge_features, W_kernel,
                           W_bias, We_kernel, We_bias, a, senders)
    nc = _build(plan, n_pad, in_dim, edge_dim, embed, heads)
    res = run_bass_kernel_spmd(nc, in_maps, core_ids=list(range(NCORES)),
                               trace=trace)
    # reassemble: core outputs are [npos*P, embed]; position rows -> blocks
    out = np.zeros((n_pad, embed), np.float32)
    for core in range(NCORES):
        o = res.results[core]["out"]
        for pos in range(plan["npos"]):
            b = int(plan["blocks"][pos, core])
            out[b * P:(b + 1) * P] = o[pos * P:(pos + 1) * P]
    out = out[:n_nodes]
    # nodes with no incoming edges: reference segment_sum gives exactly 0
    deg = np.bincount(receivers.astype(np.int64), minlength=n_nodes)
    if (deg == 0).any():
        out[deg == 0] = 0.0
    return out, res


def kernel(node_features, edge_features, W_kernel, W_bias, We_kernel,
           We_bias, a, senders, receivers):
    node_features = np.asarray(node_features, np.float32)
    edge_features = np.asarray(edge_features, np.float32)
    W_kernel = np.asarray(W_kernel, np.float32)
    W_bias = np.asarray(W_bias, np.float32)
    We_kernel = np.asarray(We_kernel, np.float32)
    We_bias = np.asarray(We_bias, np.float32)
    a = np.asarray(a, np.float32)
    senders = np.asarray(senders, np.int32)
    receivers = np.asarray(receivers, np.int32)
    out, _ = _run(node_features, edge_features, W_kernel, W_bias, We_kernel,
                  We_bias, a, senders, receivers)
    return out
